# revision 23
# baseline (speedup 1.0000x reference)
"""Trainium2 Bass kernel for nn_MultiHeadAttention_33088428048411.

B=4, S=2048, E=1024, H=16, DH=64.  Outputs: x [B,S,E], weights [H,B,S,S],
electrode_attention [B,S].

Sharding: 8 cores = (batch b in 0..3) x (head-group hg in 0..1); each core owns
one batch element and 8 heads.  Per core, on device: hidden is transposed via
the PE (hT, [e, s]); qT/kT projections land in [d, s] layout and v in [s, d];
scores are computed in both orientations ([sq, sk] for the softmax/weights
output, [sk, sq] for the attn.v contraction); softmax uses exp with the
activation accum_out row-sum (scores are O(1), no max subtraction needed);
the weights output is normalized in place on GpSimd; attn.v accumulates
unnormalized and is renormalized per query via a DVE column-broadcast of 1/Z
transposed on the PE; x_partial = attnT.T @ Wo.T for the core's 8 heads.
Host sums the two x partials per batch, adds bo, and reduces
electrode_attention from the returned weights.

All matmuls run in float32r (fp32 operands rounded by the producing DVE op;
measured ~1.4e-4 max rel err vs ~2.3e-3 for bf16, at full PE speed for
moving dims >= 256).
"""

import numpy as np
import orjson

import concourse.bass as bass
import concourse.mybir as mybir
import concourse.tile as tile
from concourse.masks import make_identity
from concourse.bass_utils import run_bass_kernel_spmd

F32 = mybir.dt.float32
F32R = mybir.dt.float32r
AF = mybir.ActivationFunctionType
MULT = mybir.AluOpType.mult
ADD = mybir.AluOpType.add

B, S, E, H, DH = 4, 2048, 1024, 16, 64
NH = 8          # heads per core
NM = NH // 2    # head pairs per core
EC = E // 128   # 8 contraction chunks
SC = S // 128   # 16 s chunks
N_CORES = 8
SCALE = 1.0 / 8.0  # 1/sqrt(DH)

# ---------------------------------------------------------------------------
# Walrus in this container rejects instructions carrying more than one sync
# wait ("Too many sync wait commands" -- the fused Matmult word has a single
# wait slot).  Tile's sem assignment attaches several.  Fix at the BIR-JSON
# level: every instruction keeps its last wait; the rest move to NoOps
# inserted immediately before it on the same engine.
_wsplit_counter = [0]


def _split_waits(module):
    for fn in module.get("functions", []):
        for bb in fn.get("blocks", []):
            out = []
            for inst in bb.get("instructions", []):
                si = inst.get("sync_info")
                waits = si.get("on_wait") if si else None
                if waits and len(waits) > 1:
                    excess, keep = waits[:-1], waits[-1:]
                    for w in excess:
                        _wsplit_counter[0] += 1
                        out.append({
                            "debug": inst.get("debug", 0),
                            "engine": inst["engine"],
                            "ins": [],
                            "name": f"{inst['name']}-ws{_wsplit_counter[0]}",
                            "opcode": "NoOp",
                            "outs": [],
                            "sync_info": {"on_update": [], "on_wait": [w]},
                        })
                    si["on_wait"] = keep
                out.append(inst)
            bb["instructions"] = out
    return module


def _install_birfix():
    if getattr(bass.Bass, "_birfix_installed", False):
        return
    orig = bass.Bass.to_json_bytes

    def to_json_bytes(self):
        return orjson.dumps(_split_waits(orjson.loads(orig(self))))

    bass.Bass.to_json_bytes = to_json_bytes
    bass.Bass._birfix_installed = True


# ---------------------------------------------------------------------------
def _load_hT_oct(nc, st_pool, hidtv, oct_, hT8):
    """Fill hT8 [128, EC, 256] from the host-transposed hidden (f32r round
    on DVE).  hidtv is hidt viewed as [c, p, s]."""
    hstage = st_pool.tile([128, EC, 256], F32, tag="hstage", name="hstage")
    nc.sync.dma_start(
        out=hstage[:],
        in_=hidtv[:, :, oct_ * 256:(oct_ + 1) * 256].rearrange(
            "c p s -> p c s"))
    nc.vector.tensor_copy(out=hT8[:], in_=hstage[:])


def _body(nc, tc, hid_d, wq_d, wk_d, wv_d, bq_d, bk_d, bv_d, wot_d,
          w_out, x_out):
    persist = tc.alloc_tile_pool(name="persist", bufs=1)
    small = tc.alloc_tile_pool(name="small", bufs=4)

    # --- constants -------------------------------------------------------
    ident = persist.tile([128, 128], F32, tag="ident", name="ident")
    make_identity(nc, ident[:])
    ones_t = persist.tile([128, 64], F32, tag="ones_t", name="ones_t")
    nc.vector.memset(ones_t[:], 1.0)

    bq_sb = persist.tile([128, NM], F32, tag="bq_sb", name="bq_sb")
    bk_sb = persist.tile([128, NM], F32, tag="bk_sb", name="bk_sb")
    bv_sb = persist.tile([128, NM], F32, tag="bv_sb", name="bv_sb")
    nc.sync.dma_start(out=bq_sb[:], in_=bq_d)
    nc.sync.dma_start(out=bk_sb[:], in_=bk_d)
    nc.sync.dma_start(out=bv_sb[:], in_=bv_d)

    # --- persistent activations -----------------------------------------
    qT = [persist.tile([128, S], F32R, tag=f"qT{m}", name=f"qT{m}")
          for m in range(NM)]
    kT = [persist.tile([128, S], F32R, tag=f"kT{m}", name=f"kT{m}")
          for m in range(NM)]
    v8 = [persist.tile([128, NH * DH], F32R, tag=f"v8_{c}", name=f"v8_{c}")
          for c in range(SC)]
    rzall = [persist.tile([128, SC], F32, tag=f"rz{h}", name=f"rz{h}")
             for h in range(NH)]

    hidtv = hid_d.rearrange("(c p) s -> c p s", p=128)

    # --- helper: one (head, sq-chunk) of scores->softmax->weights --------
    def a_iter(ps, wtp, m, hh, sqc, psa_bufs=1):
        h = 2 * m + hh
        hoff = hh * 64
        lq = qT[m][hoff:hoff + 64, sqc * 128:(sqc + 1) * 128]
        wt = wtp.tile([128, S], F32, tag="wt", name="wt")
        zs = []
        for skh in range(2):
            psa = ps.tile([128, 1024], F32, tag="psa", name="psa",
                          bufs=psa_bufs)
            for j in range(2):
                skb = skh * 2 + j
                nc.tensor.matmul(
                    psa[:, j * 512:(j + 1) * 512], lq,
                    kT[m][hoff:hoff + 64, skb * 512:(skb + 1) * 512],
                    start=True, stop=True)
            z = small.tile([128, 1], F32, tag=f"z{skh}", name=f"z{skh}")
            nc.scalar.activation(
                out=wt[:, skh * 1024:(skh + 1) * 1024], in_=psa[:],
                func=AF.Exp, scale=SCALE, accum_out=z[:])
            zs.append(z)
        zt = small.tile([128, 1], F32, tag="zt", name="zt")
        nc.vector.tensor_tensor(out=zt[:], in0=zs[0][:], in1=zs[1][:],
                                op=ADD)
        rz = rzall[h][:, sqc:sqc + 1]
        nc.vector.reciprocal(out=rz, in_=zt[:])
        # normalize halves on GpSimd + DVE, then one contiguous 1 MB DMA
        nc.gpsimd.tensor_scalar_mul(wt[:, 0:1024], wt[:, 0:1024], rz)
        nc.vector.tensor_scalar_mul(wt[:, 1024:2048], wt[:, 1024:2048], rz)
        nc.sync.dma_start(out=w_out[h, sqc * 128:(sqc + 1) * 128, :],
                          in_=wt[:])

    # =====================================================================
    # Prologue: weight rounding, kT pass, then per-oct {qT, v, pair-0 A}.
    # Pair-0's softmax starts as soon as kT is complete and its qT chunk
    # exists, so ACT ramps ~30us in instead of waiting for all projections.
    # =====================================================================
    wpk = tc.alloc_tile_pool(name="wpk", bufs=1)
    wpqv = tc.alloc_tile_pool(name="wpqv", bufs=1)
    wph = tc.alloc_tile_pool(name="wph", bufs=1)
    st1 = tc.alloc_tile_pool(name="st1", bufs=1)
    sb_w0 = tc.alloc_tile_pool(name="sb_w0", bufs=3)
    wst = tc.alloc_tile_pool(name="wst", bufs=2)
    ps1 = tc.alloc_tile_pool(name="ps1", bufs=2, space="PSUM")
    psA0 = tc.alloc_tile_pool(name="psA0", bufs=1, space="PSUM")

    wk_r = wpk.tile([128, EC, NH * DH], F32R, tag="wk_r", name="wk_r")
    wq_r = wpqv.tile([128, EC, NH * DH], F32R, tag="wq_r", name="wq_r")
    wv_r = wpqv.tile([128, EC, NH * DH], F32R, tag="wv_r", name="wv_r")
    for (src_d, dst) in ((wk_d, wk_r), (wq_d, wq_r), (wv_d, wv_r)):
        srcv = src_d.rearrange("(c p) n -> c p n", p=128)
        for c in range(EC):
            st = wst.tile([128, NH * DH], F32, tag="wstage", name="wstage")
            nc.sync.dma_start(out=st[:], in_=srcv[c])
            nc.vector.tensor_copy(out=dst[:, c, :], in_=st[:])

    # pass 1: kT for all pairs
    for oct_ in range(8):
        hT8 = wph.tile([128, EC, 256], F32R, tag="hT8", name="hT8", bufs=2)
        _load_hT_oct(nc, st1, hidtv, oct_, hT8)
        s_lo = oct_ * 256
        for m in range(NM):
            pk = ps1.tile([128, 256], F32, tag="pj", name="pk")
            for ec in range(EC):
                nc.tensor.matmul(pk[:], wk_r[:, ec, m * 128:(m + 1) * 128],
                                 hT8[:, ec, :], start=(ec == 0),
                                 stop=(ec == EC - 1))
            nc.vector.tensor_scalar_add(
                kT[m][:, s_lo:s_lo + 256], pk[:], bk_sb[:, m:m + 1])

    # pass 2: qT + v, with pair-0 A-iters interleaved per oct
    for oct_ in range(8):
        hT8 = wph.tile([128, EC, 256], F32R, tag="hT8", name="hT8", bufs=2)
        _load_hT_oct(nc, st1, hidtv, oct_, hT8)
        s_lo = oct_ * 256
        for m in range(NM):
            pq = ps1.tile([128, 256], F32, tag="pj", name="pq")
            for ec in range(EC):
                nc.tensor.matmul(pq[:], wq_r[:, ec, m * 128:(m + 1) * 128],
                                 hT8[:, ec, :], start=(ec == 0),
                                 stop=(ec == EC - 1))
            nc.vector.tensor_scalar_add(
                qT[m][:, s_lo:s_lo + 256], pq[:], bq_sb[:, m:m + 1])
        for i in range(2):
            sc = oct_ * 2 + i
            pv = ps1.tile([128, 512], F32, tag="pv", name="pv")
            for ec in range(EC):
                nc.tensor.matmul(pv[:], hT8[:, ec, i * 128:(i + 1) * 128],
                                 wv_r[:, ec, :],
                                 start=(ec == 0), stop=(ec == EC - 1))
            nc.vector.tensor_copy(out=v8[sc][:], in_=pv[:])
        for i in range(2):
            sqc = oct_ * 2 + i
            for hh in range(2):
                a_iter(psA0, sb_w0, 0, hh, sqc, psa_bufs=2)

    psA0.release()
    ps1.release()
    wst.release()
    sb_w0.release()
    st1.release()
    wph.release()
    wpqv.release()
    wpk.release()

    # =====================================================================
    # Attention pipeline: T(m) || A(m+1), then T(3) || out-projection.
    # =====================================================================
    pb_attn = tc.alloc_tile_pool(name="pb_attn", bufs=1)
    sb_w = tc.alloc_tile_pool(name="sb_w", bufs=3)
    sb_exp = tc.alloc_tile_pool(name="sb_exp", bufs=2)
    sb_bc = tc.alloc_tile_pool(name="sb_bc", bufs=2)
    attnT = [pb_attn.tile([128, S], F32R, tag=f"attnT{m}", name=f"attnT{m}")
             for m in range(NM)]

    def t_block(ps, m, sqb, filler=None, pt_bufs=1, acc_bufs=2):
        acc0 = ps.tile([64, 512], F32, tag="acc0", name="acc0",
                       bufs=acc_bufs)
        acc1 = ps.tile([64, 512], F32, tag="acc1", name="acc1",
                       bufs=acc_bufs)
        rq0 = qT[m][0:64, sqb * 512:(sqb + 1) * 512]
        rq1 = qT[m][64:128, sqb * 512:(sqb + 1) * 512]
        for skc in range(SC):
            pt = ps.tile([128, 1024], F32, tag="pt", name="pt",
                         bufs=pt_bufs)
            nc.tensor.matmul(pt[:, 0:512],
                             kT[m][0:64, skc * 128:(skc + 1) * 128],
                             rq0, start=True, stop=True,
                             tile_position=(0, 0))
            nc.tensor.matmul(pt[:, 512:1024],
                             kT[m][64:128, skc * 128:(skc + 1) * 128],
                             rq1, start=True, stop=True,
                             tile_position=(64, 0))
            e = sb_exp.tile([128, 1024], F32R, tag="e", name="e")
            nc.scalar.activation(out=e[:], in_=pt[:], func=AF.Exp,
                                 scale=SCALE)
            nc.tensor.matmul(acc0[:], v8[skc][:, m * 128:m * 128 + 64],
                             e[:, 0:512], start=(skc == 0),
                             stop=(skc == SC - 1))
            nc.tensor.matmul(acc1[:],
                             v8[skc][:, m * 128 + 64:m * 128 + 128],
                             e[:, 512:1024], start=(skc == 0),
                             stop=(skc == SC - 1))
            if filler is not None:
                filler(skc)
        for hh, acc in ((0, acc0), (1, acc1)):
            h = 2 * m + hh
            pbc = ps.tile([64, 512], F32, tag="pt", name=f"pbc{hh}",
                          bufs=pt_bufs)
            for c in range(4):
                sqc = sqb * 4 + c
                cb = sb_bc.tile([128, 64], F32, tag="cb", name="cb")
                nc.vector.tensor_scalar_mul(cb[:], ones_t[:],
                                            rzall[h][:, sqc:sqc + 1])
                nc.tensor.transpose(pbc[:, c * 128:(c + 1) * 128], cb[:],
                                    ident[:])
            pbc_sb = sb_bc.tile([64, 512], F32, tag="pbc_sb",
                                name="pbc_sb")
            nc.vector.tensor_copy(out=pbc_sb[:], in_=pbc[:])
            dst = attnT[m][hh * 64:(hh + 1) * 64,
                           sqb * 512:(sqb + 1) * 512]
            nc.vector.tensor_tensor(out=dst, in0=acc[:], in1=pbc_sb[:],
                                    op=MULT)
            nc.vector.tensor_scalar_add(
                dst, dst, bv_sb[hh * 64:(hh + 1) * 64, m:m + 1])

    with tc.tile_pool(name="psAT", bufs=1, space="PSUM") as ps:
        for m in range(NM - 1):
            for sqb in range(4):
                for sqc4 in range(4):
                    sqc = sqb * 4 + sqc4
                    for hh in range(2):
                        a_iter(ps, sb_w, m + 1, hh, sqc)
                t_block(ps, m, sqb)

    # ---- T(3) overlapped with the output projection (lagged 1 block) ----
    with tc.tile_pool(name="psF", bufs=1, space="PSUM") as psF, \
         tc.tile_pool(name="wp3", bufs=1) as wp3, \
         tc.tile_pool(name="st3", bufs=1) as st3:
        wot_r = wp3.tile([128, NM, E], F32R, tag="wot_r", name="wot_r")
        wotv = wot_d.rearrange("(c p) n -> c p n", p=128)
        for c in range(NM):
            st = st3.tile([128, E], F32, tag="wotstage", name="wotstage")
            nc.sync.dma_start(out=st[:], in_=wotv[c])
            nc.vector.tensor_copy(out=wot_r[:, c, :], in_=st[:])

        xt_cur = [None]

        def outproj_piece(sqb, g):
            sqc4, eb = divmod(g, 2)
            sqc = sqb * 4 + sqc4
            if eb == 0:
                xt_cur[0] = sb_w.tile([128, E], F32, tag="wt", name="xt")
            xt = xt_cur[0]
            px = psF.tile([128, 512], F32, tag="px", name="px", bufs=2)
            for cc in range(NM):
                nc.tensor.matmul(
                    px[:], attnT[cc][:, sqc * 128:(sqc + 1) * 128],
                    wot_r[:, cc, eb * 512:(eb + 1) * 512],
                    start=(cc == 0), stop=(cc == NM - 1))
            nc.vector.tensor_copy(out=xt[:, eb * 512:(eb + 1) * 512],
                                  in_=px[:])
            if eb == 1:
                nc.sync.dma_start(out=x_out[sqc * 128:(sqc + 1) * 128, :],
                                  in_=xt[:])

        for sqb in range(4):
            if sqb > 0:
                fill = lambda skc, b=sqb - 1: (
                    outproj_piece(b, skc // 2) if skc % 2 == 1 else None)
            else:
                fill = None
            t_block(psF, 3, sqb, filler=fill, pt_bufs=2, acc_bufs=1)
        for g in range(8):
            outproj_piece(3, g)

    sb_bc.release()
    sb_exp.release()
    sb_w.release()
    pb_attn.release()
    small.release()
    persist.release()


def _build_bass():
    nc = bass.Bass("TRN2", target_bir_lowering=False, debug=False,
                   num_devices=N_CORES)

    hid_d = nc.dram_tensor("hidt", [E, S], F32, kind="ExternalInput").ap()
    wq_d = nc.dram_tensor("wq", [E, NH * DH], F32, kind="ExternalInput").ap()
    wk_d = nc.dram_tensor("wk", [E, NH * DH], F32, kind="ExternalInput").ap()
    wv_d = nc.dram_tensor("wv", [E, NH * DH], F32, kind="ExternalInput").ap()
    bq_d = nc.dram_tensor("bq", [128, NM], F32, kind="ExternalInput").ap()
    bk_d = nc.dram_tensor("bk", [128, NM], F32, kind="ExternalInput").ap()
    bv_d = nc.dram_tensor("bv", [128, NM], F32, kind="ExternalInput").ap()
    wot_d = nc.dram_tensor("wot", [NH * DH, E], F32, kind="ExternalInput").ap()

    w_out = nc.dram_tensor("w_out", [NH, S, S], F32,
                           kind="ExternalOutput").ap()
    x_out = nc.dram_tensor("x_out", [S, E], F32, kind="ExternalOutput").ap()

    with tile.TileContext(nc) as tc:
        _body(nc, tc, hid_d, wq_d, wk_d, wv_d, bq_d, bk_d, bv_d, wot_d,
              w_out, x_out)
    return nc


_nc_cache = [None]


def _get_nc():
    if _nc_cache[0] is None:
        _install_birfix()
        _nc_cache[0] = _build_bass()
    return _nc_cache[0]


def kernel(hidden_state, Wq, bq, Wk, bk, Wv, bv, Wo, bo, _want_trace=False):
    hidden_state = np.asarray(hidden_state, dtype=np.float32)
    Wq = np.asarray(Wq, dtype=np.float32)
    Wk = np.asarray(Wk, dtype=np.float32)
    Wv = np.asarray(Wv, dtype=np.float32)
    bq = np.asarray(bq, dtype=np.float32)
    bk = np.asarray(bk, dtype=np.float32)
    bv = np.asarray(bv, dtype=np.float32)
    Wo = np.asarray(Wo, dtype=np.float32)
    bo = np.asarray(bo, dtype=np.float32)

    nc = _get_nc()

    in_maps = []
    for core in range(N_CORES):
        b = core // 2
        hg = core % 2
        hs = slice(hg * NH, (hg + 1) * NH)
        # [h, E, DH] -> [E, h*DH]  (head-major feature order)
        wq8 = np.ascontiguousarray(
            Wq[hs].transpose(1, 0, 2).reshape(E, NH * DH))
        wk8 = np.ascontiguousarray(
            Wk[hs].transpose(1, 0, 2).reshape(E, NH * DH))
        wv8 = np.ascontiguousarray(
            Wv[hs].transpose(1, 0, 2).reshape(E, NH * DH))
        # biases as [128, NM]: partition = pair-local c (2 heads x 64 d)
        bq8 = np.ascontiguousarray(bq[hs].reshape(NM, 128).T)
        bk8 = np.ascontiguousarray(bk[hs].reshape(NM, 128).T)
        bv8 = np.ascontiguousarray(bv[hs].reshape(NM, 128).T)
        # Wo column slice, transposed: [c, eo]
        wot = np.ascontiguousarray(Wo[:, hg * 512:(hg + 1) * 512].T)
        in_maps.append({
            "hidt": np.ascontiguousarray(hidden_state[b].T),
            "wq": wq8, "wk": wk8, "wv": wv8,
            "bq": bq8, "bk": bk8, "bv": bv8,
            "wot": wot,
        })

    res = run_bass_kernel_spmd(nc, in_maps, list(range(N_CORES)),
                               trace=_want_trace)

    weights = np.empty((H, B, S, S), dtype=np.float32)
    x = np.zeros((B, S, E), dtype=np.float64)
    for core in range(N_CORES):
        b = core // 2
        hg = core % 2
        r = res.results[core]
        weights[hg * NH:(hg + 1) * NH, b] = r["w_out"]
        x[b] += r["x_out"].astype(np.float64)
    x += bo.astype(np.float64)
    x = x.astype(np.float32)

    # electrode_attention = mean over heads then mean over query dim
    electrode = weights.astype(np.float64).mean(axis=(0, 2)).astype(np.float32)

    if _want_trace:
        kernel._last_result = res
    return x, weights, electrode


# revision 24
# speedup vs baseline: 49.7299x; 49.7299x over previous
"""Trainium2 Bass kernel for nn_MultiHeadAttention_33088428048411.

B=4, S=2048, E=1024, H=16, DH=64.  Outputs: x [B,S,E], weights [H,B,S,S],
electrode_attention [B,S].

Sharding: 8 cores = (batch b in 0..3) x (head-group hg in 0..1); each core owns
one batch element and 8 heads.  Per core, on device: hidden is transposed via
the PE (hT, [e, s]); qT/kT projections land in [d, s] layout and v in [s, d];
scores are computed in both orientations ([sq, sk] for the softmax/weights
output, [sk, sq] for the attn.v contraction); softmax uses exp with the
activation accum_out row-sum (scores are O(1), no max subtraction needed);
the weights output is normalized in place on GpSimd; attn.v accumulates
unnormalized and is renormalized per query via a DVE column-broadcast of 1/Z
transposed on the PE; x_partial = attnT.T @ Wo.T for the core's 8 heads.
Host sums the two x partials per batch, adds bo, and reduces
electrode_attention from the returned weights.

All matmuls run in float32r (fp32 operands rounded by the producing DVE op;
measured ~1.4e-4 max rel err vs ~2.3e-3 for bf16, at full PE speed for
moving dims >= 256).
"""

import numpy as np
import orjson

import concourse.bass as bass
import concourse.mybir as mybir
import concourse.tile as tile
from concourse.masks import make_identity
from concourse.bass_utils import run_bass_kernel_spmd

F32 = mybir.dt.float32
F32R = mybir.dt.float32r
AF = mybir.ActivationFunctionType
MULT = mybir.AluOpType.mult
ADD = mybir.AluOpType.add

B, S, E, H, DH = 4, 2048, 1024, 16, 64
NH = 8          # heads per core
NM = NH // 2    # head pairs per core
EC = E // 128   # 8 contraction chunks
SC = S // 128   # 16 s chunks
N_CORES = 8
SCALE = 1.0 / 8.0  # 1/sqrt(DH)

# ---------------------------------------------------------------------------
# Walrus in this container rejects instructions carrying more than one sync
# wait ("Too many sync wait commands" -- the fused Matmult word has a single
# wait slot).  Tile's sem assignment attaches several.  Fix at the BIR-JSON
# level: every instruction keeps its last wait; the rest move to NoOps
# inserted immediately before it on the same engine.
_wsplit_counter = [0]


def _split_waits(module):
    for fn in module.get("functions", []):
        for bb in fn.get("blocks", []):
            out = []
            for inst in bb.get("instructions", []):
                si = inst.get("sync_info")
                waits = si.get("on_wait") if si else None
                if waits and len(waits) > 1:
                    excess, keep = waits[:-1], waits[-1:]
                    for w in excess:
                        _wsplit_counter[0] += 1
                        out.append({
                            "debug": inst.get("debug", 0),
                            "engine": inst["engine"],
                            "ins": [],
                            "name": f"{inst['name']}-ws{_wsplit_counter[0]}",
                            "opcode": "NoOp",
                            "outs": [],
                            "sync_info": {"on_update": [], "on_wait": [w]},
                        })
                    si["on_wait"] = keep
                out.append(inst)
            bb["instructions"] = out
    return module


def _install_birfix():
    if getattr(bass.Bass, "_birfix_installed", False):
        return
    orig = bass.Bass.to_json_bytes

    def to_json_bytes(self):
        return orjson.dumps(_split_waits(orjson.loads(orig(self))))

    bass.Bass.to_json_bytes = to_json_bytes
    bass.Bass._birfix_installed = True


# ---------------------------------------------------------------------------
def _load_hT_oct(nc, st_pool, hidtv, oct_, hT8):
    """Fill hT8 [128, EC, 256] from the host-transposed hidden (f32r round
    on DVE).  hidtv is hidt viewed as [c, p, s]."""
    hstage = st_pool.tile([128, EC, 256], F32, tag="hstage", name="hstage")
    nc.sync.dma_start(
        out=hstage[:],
        in_=hidtv[:, :, oct_ * 256:(oct_ + 1) * 256].rearrange(
            "c p s -> p c s"))
    nc.vector.tensor_copy(out=hT8[:], in_=hstage[:])


def _body(nc, tc, hid_d, wq_d, wk_d, wv_d, bq_d, bk_d, bv_d, wot_d,
          w_out, x_out):
    persist = tc.alloc_tile_pool(name="persist", bufs=1)
    small = tc.alloc_tile_pool(name="small", bufs=4)

    # --- constants -------------------------------------------------------
    ident = persist.tile([128, 128], F32, tag="ident", name="ident")
    make_identity(nc, ident[:])
    ones_t = persist.tile([128, 64], F32, tag="ones_t", name="ones_t")
    nc.vector.memset(ones_t[:], 1.0)

    bq_sb = persist.tile([128, NM], F32, tag="bq_sb", name="bq_sb")
    bk_sb = persist.tile([128, NM], F32, tag="bk_sb", name="bk_sb")
    bv_sb = persist.tile([128, NM], F32, tag="bv_sb", name="bv_sb")
    nc.sync.dma_start(out=bq_sb[:], in_=bq_d)
    nc.sync.dma_start(out=bk_sb[:], in_=bk_d)
    nc.sync.dma_start(out=bv_sb[:], in_=bv_d)

    # --- persistent activations -----------------------------------------
    qT = [persist.tile([128, S], F32R, tag=f"qT{m}", name=f"qT{m}")
          for m in range(NM)]
    kT = [persist.tile([128, S], F32R, tag=f"kT{m}", name=f"kT{m}")
          for m in range(NM)]
    v8 = [persist.tile([128, NH * DH], F32R, tag=f"v8_{c}", name=f"v8_{c}")
          for c in range(SC)]
    rzall = [persist.tile([128, SC], F32, tag=f"rz{h}", name=f"rz{h}")
             for h in range(NH)]

    hidtv = hid_d.rearrange("(c p) s -> c p s", p=128)

    # --- helper: one (head, sq-chunk) of scores->softmax->weights --------
    def a_iter(ps, wtp, m, hh, sqc, psa_bufs=1):
        h = 2 * m + hh
        hoff = hh * 64
        lq = qT[m][hoff:hoff + 64, sqc * 128:(sqc + 1) * 128]
        wt = wtp.tile([128, S], F32, tag="wt", name="wt")
        zs = []
        for skh in range(2):
            psa = ps.tile([128, 1024], F32, tag="psa", name="psa",
                          bufs=psa_bufs)
            for j in range(2):
                skb = skh * 2 + j
                nc.tensor.matmul(
                    psa[:, j * 512:(j + 1) * 512], lq,
                    kT[m][hoff:hoff + 64, skb * 512:(skb + 1) * 512],
                    start=True, stop=True)
            z = small.tile([128, 1], F32, tag=f"z{skh}", name=f"z{skh}")
            nc.scalar.activation(
                out=wt[:, skh * 1024:(skh + 1) * 1024], in_=psa[:],
                func=AF.Exp, scale=SCALE, accum_out=z[:])
            zs.append(z)
        zt = small.tile([128, 1], F32, tag="zt", name="zt")
        nc.vector.tensor_tensor(out=zt[:], in0=zs[0][:], in1=zs[1][:],
                                op=ADD)
        rz = rzall[h][:, sqc:sqc + 1]
        nc.vector.reciprocal(out=rz, in_=zt[:])
        # normalize halves on GpSimd + DVE, then one contiguous 1 MB DMA
        nc.gpsimd.tensor_scalar_mul(wt[:, 0:1024], wt[:, 0:1024], rz)
        nc.vector.tensor_scalar_mul(wt[:, 1024:2048], wt[:, 1024:2048], rz)
        nc.sync.dma_start(out=w_out[h, sqc * 128:(sqc + 1) * 128, :],
                          in_=wt[:])

    # =====================================================================
    # Prologue: weight rounding, kT pass, then per-oct {qT, v, pair-0 A}.
    # Pair-0's softmax starts as soon as kT is complete and its qT chunk
    # exists, so ACT ramps ~30us in instead of waiting for all projections.
    # =====================================================================
    wpk = tc.alloc_tile_pool(name="wpk", bufs=1)
    wpqv = tc.alloc_tile_pool(name="wpqv", bufs=1)
    wph = tc.alloc_tile_pool(name="wph", bufs=1)
    st1 = tc.alloc_tile_pool(name="st1", bufs=1)
    sb_w0 = tc.alloc_tile_pool(name="sb_w0", bufs=3)
    wst = tc.alloc_tile_pool(name="wst", bufs=2)
    ps1 = tc.alloc_tile_pool(name="ps1", bufs=2, space="PSUM")
    psA0 = tc.alloc_tile_pool(name="psA0", bufs=1, space="PSUM")

    wk_r = wpk.tile([128, EC, NH * DH], F32R, tag="wk_r", name="wk_r")
    wq_r = wpqv.tile([128, EC, NH * DH], F32R, tag="wq_r", name="wq_r")
    wv_r = wpqv.tile([128, EC, NH * DH], F32R, tag="wv_r", name="wv_r")
    for (src_d, dst) in ((wk_d, wk_r), (wq_d, wq_r), (wv_d, wv_r)):
        srcv = src_d.rearrange("(c p) n -> c p n", p=128)
        for c in range(EC):
            st = wst.tile([128, NH * DH], F32, tag="wstage", name="wstage")
            nc.sync.dma_start(out=st[:], in_=srcv[c])
            nc.vector.tensor_copy(out=dst[:, c, :], in_=st[:])

    # pass 1: kT for all pairs
    for oct_ in range(8):
        hT8 = wph.tile([128, EC, 256], F32R, tag="hT8", name="hT8", bufs=2)
        _load_hT_oct(nc, st1, hidtv, oct_, hT8)
        s_lo = oct_ * 256
        for m in range(NM):
            pk = ps1.tile([128, 256], F32, tag="pj", name="pk")
            for ec in range(EC):
                nc.tensor.matmul(pk[:], wk_r[:, ec, m * 128:(m + 1) * 128],
                                 hT8[:, ec, :], start=(ec == 0),
                                 stop=(ec == EC - 1))
            nc.vector.tensor_scalar_add(
                kT[m][:, s_lo:s_lo + 256], pk[:], bk_sb[:, m:m + 1])

    # pass 2: qT + v, with pair-0 A-iters interleaved per oct
    for oct_ in range(8):
        hT8 = wph.tile([128, EC, 256], F32R, tag="hT8", name="hT8", bufs=2)
        _load_hT_oct(nc, st1, hidtv, oct_, hT8)
        s_lo = oct_ * 256
        for m in range(NM):
            pq = ps1.tile([128, 256], F32, tag="pj", name="pq")
            for ec in range(EC):
                nc.tensor.matmul(pq[:], wq_r[:, ec, m * 128:(m + 1) * 128],
                                 hT8[:, ec, :], start=(ec == 0),
                                 stop=(ec == EC - 1))
            nc.vector.tensor_scalar_add(
                qT[m][:, s_lo:s_lo + 256], pq[:], bq_sb[:, m:m + 1])
        for i in range(2):
            sc = oct_ * 2 + i
            pv = ps1.tile([128, 512], F32, tag="pv", name="pv")
            for ec in range(EC):
                nc.tensor.matmul(pv[:], hT8[:, ec, i * 128:(i + 1) * 128],
                                 wv_r[:, ec, :],
                                 start=(ec == 0), stop=(ec == EC - 1))
            nc.vector.tensor_copy(out=v8[sc][:], in_=pv[:])
        for i in range(2):
            sqc = oct_ * 2 + i
            for hh in range(2):
                a_iter(psA0, sb_w0, 0, hh, sqc, psa_bufs=2)

    psA0.release()
    ps1.release()
    wst.release()
    sb_w0.release()
    st1.release()
    wph.release()
    wpqv.release()
    wpk.release()

    # =====================================================================
    # Attention pipeline: T(m) || A(m+1), then T(3) || out-projection.
    # =====================================================================
    pb_attn = tc.alloc_tile_pool(name="pb_attn", bufs=1)
    sb_w = tc.alloc_tile_pool(name="sb_w", bufs=3)
    sb_exp = tc.alloc_tile_pool(name="sb_exp", bufs=2)
    sb_bc = tc.alloc_tile_pool(name="sb_bc", bufs=2)
    attnT = [pb_attn.tile([128, S], F32R, tag=f"attnT{m}", name=f"attnT{m}")
             for m in range(NM)]

    def t_block(ps, m, sqb, filler=None, pt_bufs=1, acc_bufs=2):
        acc0 = ps.tile([64, 512], F32, tag="acc0", name="acc0",
                       bufs=acc_bufs)
        acc1 = ps.tile([64, 512], F32, tag="acc1", name="acc1",
                       bufs=acc_bufs)
        rq0 = qT[m][0:64, sqb * 512:(sqb + 1) * 512]
        rq1 = qT[m][64:128, sqb * 512:(sqb + 1) * 512]
        for skc in range(SC):
            pt = ps.tile([128, 1024], F32, tag="pt", name="pt",
                         bufs=pt_bufs)
            nc.tensor.matmul(pt[:, 0:512],
                             kT[m][0:64, skc * 128:(skc + 1) * 128],
                             rq0, start=True, stop=True,
                             tile_position=(0, 0))
            nc.tensor.matmul(pt[:, 512:1024],
                             kT[m][64:128, skc * 128:(skc + 1) * 128],
                             rq1, start=True, stop=True,
                             tile_position=(64, 0))
            e = sb_exp.tile([128, 1024], F32R, tag="e", name="e")
            nc.scalar.activation(out=e[:], in_=pt[:], func=AF.Exp,
                                 scale=SCALE)
            nc.tensor.matmul(acc0[:], v8[skc][:, m * 128:m * 128 + 64],
                             e[:, 0:512], start=(skc == 0),
                             stop=(skc == SC - 1))
            nc.tensor.matmul(acc1[:],
                             v8[skc][:, m * 128 + 64:m * 128 + 128],
                             e[:, 512:1024], start=(skc == 0),
                             stop=(skc == SC - 1))
            if filler is not None:
                filler(skc)
        for hh, acc in ((0, acc0), (1, acc1)):
            h = 2 * m + hh
            pbc = ps.tile([64, 512], F32, tag="pt", name=f"pbc{hh}",
                          bufs=pt_bufs)
            for c in range(4):
                sqc = sqb * 4 + c
                cb = sb_bc.tile([128, 64], F32, tag="cb", name="cb")
                nc.vector.tensor_scalar_mul(cb[:], ones_t[:],
                                            rzall[h][:, sqc:sqc + 1])
                nc.tensor.transpose(pbc[:, c * 128:(c + 1) * 128], cb[:],
                                    ident[:])
            pbc_sb = sb_bc.tile([64, 512], F32, tag="pbc_sb",
                                name="pbc_sb")
            nc.vector.tensor_copy(out=pbc_sb[:], in_=pbc[:])
            dst = attnT[m][hh * 64:(hh + 1) * 64,
                           sqb * 512:(sqb + 1) * 512]
            nc.vector.tensor_tensor(out=dst, in0=acc[:], in1=pbc_sb[:],
                                    op=MULT)
            nc.vector.tensor_scalar_add(
                dst, dst, bv_sb[hh * 64:(hh + 1) * 64, m:m + 1])

    with tc.tile_pool(name="psAT", bufs=1, space="PSUM") as ps:
        for m in range(NM - 1):
            for sqb in range(4):
                for sqc4 in range(4):
                    sqc = sqb * 4 + sqc4
                    for hh in range(2):
                        a_iter(ps, sb_w, m + 1, hh, sqc)
                t_block(ps, m, sqb)

    # ---- T(3) overlapped with the output projection (lagged 1 block) ----
    with tc.tile_pool(name="psF", bufs=1, space="PSUM") as psF, \
         tc.tile_pool(name="wp3", bufs=1) as wp3, \
         tc.tile_pool(name="st3", bufs=1) as st3:
        wot_r = wp3.tile([128, NM, E], F32R, tag="wot_r", name="wot_r")
        wotv = wot_d.rearrange("(c p) n -> c p n", p=128)
        for c in range(NM):
            st = st3.tile([128, E], F32, tag="wotstage", name="wotstage")
            nc.sync.dma_start(out=st[:], in_=wotv[c])
            nc.vector.tensor_copy(out=wot_r[:, c, :], in_=st[:])

        xt_cur = [None]

        def outproj_piece(sqb, g):
            sqc4, eb = divmod(g, 2)
            sqc = sqb * 4 + sqc4
            if eb == 0:
                xt_cur[0] = sb_w.tile([128, E], F32, tag="wt", name="xt")
            xt = xt_cur[0]
            px = psF.tile([128, 512], F32, tag="px", name="px", bufs=2)
            for cc in range(NM):
                nc.tensor.matmul(
                    px[:], attnT[cc][:, sqc * 128:(sqc + 1) * 128],
                    wot_r[:, cc, eb * 512:(eb + 1) * 512],
                    start=(cc == 0), stop=(cc == NM - 1))
            nc.vector.tensor_copy(out=xt[:, eb * 512:(eb + 1) * 512],
                                  in_=px[:])
            if eb == 1:
                nc.sync.dma_start(out=x_out[sqc * 128:(sqc + 1) * 128, :],
                                  in_=xt[:])

        for sqb in range(4):
            if sqb > 0:
                fill = lambda skc, b=sqb - 1: (
                    outproj_piece(b, skc // 2) if skc % 2 == 1 else None)
            else:
                fill = None
            t_block(psF, 3, sqb, filler=fill, pt_bufs=2, acc_bufs=1)
        for g in range(8):
            outproj_piece(3, g)

    sb_bc.release()
    sb_exp.release()
    sb_w.release()
    pb_attn.release()
    small.release()
    persist.release()


def _build_bass(repeat=1):
    nc = bass.Bass("TRN2", target_bir_lowering=False, debug=False,
                   num_devices=N_CORES)

    hid_d = nc.dram_tensor("hidt", [E, S], F32, kind="ExternalInput").ap()
    wq_d = nc.dram_tensor("wq", [E, NH * DH], F32, kind="ExternalInput").ap()
    wk_d = nc.dram_tensor("wk", [E, NH * DH], F32, kind="ExternalInput").ap()
    wv_d = nc.dram_tensor("wv", [E, NH * DH], F32, kind="ExternalInput").ap()
    bq_d = nc.dram_tensor("bq", [128, NM], F32, kind="ExternalInput").ap()
    bk_d = nc.dram_tensor("bk", [128, NM], F32, kind="ExternalInput").ap()
    bv_d = nc.dram_tensor("bv", [128, NM], F32, kind="ExternalInput").ap()
    wot_d = nc.dram_tensor("wot", [NH * DH, E], F32, kind="ExternalInput").ap()

    w_out = nc.dram_tensor("w_out", [NH, S, S], F32,
                           kind="ExternalOutput").ap()
    x_out = nc.dram_tensor("x_out", [S, E], F32, kind="ExternalOutput").ap()

    with tile.TileContext(nc) as tc:
        for _ in range(repeat):
            _body(nc, tc, hid_d, wq_d, wk_d, wv_d, bq_d, bk_d, bv_d, wot_d,
                  w_out, x_out)
    return nc


_nc_cache = [None]


def _get_nc():
    if _nc_cache[0] is None:
        _install_birfix()
        _nc_cache[0] = _build_bass()
    return _nc_cache[0]


def kernel(hidden_state, Wq, bq, Wk, bk, Wv, bv, Wo, bo, _want_trace=False):
    hidden_state = np.asarray(hidden_state, dtype=np.float32)
    Wq = np.asarray(Wq, dtype=np.float32)
    Wk = np.asarray(Wk, dtype=np.float32)
    Wv = np.asarray(Wv, dtype=np.float32)
    bq = np.asarray(bq, dtype=np.float32)
    bk = np.asarray(bk, dtype=np.float32)
    bv = np.asarray(bv, dtype=np.float32)
    Wo = np.asarray(Wo, dtype=np.float32)
    bo = np.asarray(bo, dtype=np.float32)

    nc = _get_nc()

    in_maps = []
    for core in range(N_CORES):
        b = core // 2
        hg = core % 2
        hs = slice(hg * NH, (hg + 1) * NH)
        # [h, E, DH] -> [E, h*DH]  (head-major feature order)
        wq8 = np.ascontiguousarray(
            Wq[hs].transpose(1, 0, 2).reshape(E, NH * DH))
        wk8 = np.ascontiguousarray(
            Wk[hs].transpose(1, 0, 2).reshape(E, NH * DH))
        wv8 = np.ascontiguousarray(
            Wv[hs].transpose(1, 0, 2).reshape(E, NH * DH))
        # biases as [128, NM]: partition = pair-local c (2 heads x 64 d)
        bq8 = np.ascontiguousarray(bq[hs].reshape(NM, 128).T)
        bk8 = np.ascontiguousarray(bk[hs].reshape(NM, 128).T)
        bv8 = np.ascontiguousarray(bv[hs].reshape(NM, 128).T)
        # Wo column slice, transposed: [c, eo]
        wot = np.ascontiguousarray(Wo[:, hg * 512:(hg + 1) * 512].T)
        in_maps.append({
            "hidt": np.ascontiguousarray(hidden_state[b].T),
            "wq": wq8, "wk": wk8, "wv": wv8,
            "bq": bq8, "bk": bk8, "bv": bv8,
            "wot": wot,
        })

    res = run_bass_kernel_spmd(nc, in_maps, list(range(N_CORES)),
                               trace=_want_trace)

    weights = np.empty((H, B, S, S), dtype=np.float32)
    x = np.zeros((B, S, E), dtype=np.float64)
    for core in range(N_CORES):
        b = core // 2
        hg = core % 2
        r = res.results[core]
        weights[hg * NH:(hg + 1) * NH, b] = r["w_out"]
        x[b] += r["x_out"].astype(np.float64)
    x += bo.astype(np.float64)
    x = x.astype(np.float32)

    # electrode_attention = mean over heads then mean over query dim
    electrode = weights.astype(np.float64).mean(axis=(0, 2)).astype(np.float32)

    if _want_trace:
        kernel._last_result = res
    return x, weights, electrode


# revision 25
# speedup vs baseline: 71.8538x; 1.4449x over previous
"""Trainium2 Bass kernel for nn_MultiHeadAttention_33088428048411.

B=4, S=2048, E=1024, H=16, DH=64.  Outputs: x [B,S,E], weights [H,B,S,S],
electrode_attention [B,S].

Sharding: 8 cores = (batch b in 0..3) x (head-group hg in 0..1); each core owns
one batch element and 8 heads.  Per core, on device: hidden is transposed via
the PE (hT, [e, s]); qT/kT projections land in [d, s] layout and v in [s, d];
scores are computed in both orientations ([sq, sk] for the softmax/weights
output, [sk, sq] for the attn.v contraction); softmax uses exp with the
activation accum_out row-sum (scores are O(1), no max subtraction needed);
the weights output is normalized in place on GpSimd; attn.v accumulates
unnormalized and is renormalized per query via a DVE column-broadcast of 1/Z
transposed on the PE; x_partial = attnT.T @ Wo.T for the core's 8 heads.
Host sums the two x partials per batch, adds bo, and reduces
electrode_attention from the returned weights.

All matmuls run in float32r (fp32 operands rounded by the producing DVE op;
measured ~1.4e-4 max rel err vs ~2.3e-3 for bf16, at full PE speed for
moving dims >= 256).
"""

import numpy as np
import orjson

import concourse.bass as bass
import concourse.mybir as mybir
import concourse.tile as tile
from concourse.masks import make_identity
from concourse.bass_utils import run_bass_kernel_spmd

F32 = mybir.dt.float32
F32R = mybir.dt.float32r
AF = mybir.ActivationFunctionType
MULT = mybir.AluOpType.mult
ADD = mybir.AluOpType.add

B, S, E, H, DH = 4, 2048, 1024, 16, 64
NH = 8          # heads per core
NM = NH // 2    # head pairs per core
EC = E // 128   # 8 contraction chunks
SC = S // 128   # 16 s chunks
N_CORES = 8
SCALE = 1.0 / 8.0  # 1/sqrt(DH)

# ---------------------------------------------------------------------------
# Walrus in this container rejects instructions carrying more than one sync
# wait ("Too many sync wait commands" -- the fused Matmult word has a single
# wait slot).  Tile's sem assignment attaches several.  Fix at the BIR-JSON
# level: every instruction keeps its last wait; the rest move to NoOps
# inserted immediately before it on the same engine.
_wsplit_counter = [0]


def _split_waits(module):
    for fn in module.get("functions", []):
        for bb in fn.get("blocks", []):
            out = []
            for inst in bb.get("instructions", []):
                si = inst.get("sync_info")
                waits = si.get("on_wait") if si else None
                if waits and len(waits) > 1:
                    excess, keep = waits[:-1], waits[-1:]
                    for w in excess:
                        _wsplit_counter[0] += 1
                        out.append({
                            "debug": inst.get("debug", 0),
                            "engine": inst["engine"],
                            "ins": [],
                            "name": f"{inst['name']}-ws{_wsplit_counter[0]}",
                            "opcode": "NoOp",
                            "outs": [],
                            "sync_info": {"on_update": [], "on_wait": [w]},
                        })
                    si["on_wait"] = keep
                out.append(inst)
            bb["instructions"] = out
    return module


def _install_birfix():
    if getattr(bass.Bass, "_birfix_installed", False):
        return
    orig = bass.Bass.to_json_bytes

    def to_json_bytes(self):
        return orjson.dumps(_split_waits(orjson.loads(orig(self))))

    bass.Bass.to_json_bytes = to_json_bytes
    bass.Bass._birfix_installed = True


# ---------------------------------------------------------------------------
def _load_hT_oct(nc, st_pool, hidtv, oct_, hT8):
    """Fill hT8 [128, EC, 256] from the host-transposed hidden (f32r round
    on DVE).  hidtv is hidt viewed as [c, p, s]."""
    hstage = st_pool.tile([128, EC, 256], F32, tag="hstage", name="hstage")
    nc.sync.dma_start(
        out=hstage[:],
        in_=hidtv[:, :, oct_ * 256:(oct_ + 1) * 256].rearrange(
            "c p s -> p c s"))
    nc.vector.tensor_copy(out=hT8[:], in_=hstage[:])


def _body(nc, tc, hid_d, wq_d, wk_d, wv_d, bq_d, bk_d, bv_d, wot_d,
          w_out, x_out):
    persist = tc.alloc_tile_pool(name="persist", bufs=1)
    small = tc.alloc_tile_pool(name="small", bufs=4)

    # --- constants -------------------------------------------------------
    ident = persist.tile([128, 128], F32, tag="ident", name="ident")
    make_identity(nc, ident[:])
    ones_t = persist.tile([128, 64], F32, tag="ones_t", name="ones_t")
    nc.vector.memset(ones_t[:], 1.0)

    bq_sb = persist.tile([128, NM], F32, tag="bq_sb", name="bq_sb")
    bk_sb = persist.tile([128, NM], F32, tag="bk_sb", name="bk_sb")
    bv_sb = persist.tile([128, NM], F32, tag="bv_sb", name="bv_sb")
    nc.sync.dma_start(out=bq_sb[:], in_=bq_d)
    nc.sync.dma_start(out=bk_sb[:], in_=bk_d)
    nc.sync.dma_start(out=bv_sb[:], in_=bv_d)

    # --- persistent activations -----------------------------------------
    qT = [persist.tile([128, S], F32R, tag=f"qT{m}", name=f"qT{m}")
          for m in range(NM)]
    kT = [persist.tile([128, S], F32R, tag=f"kT{m}", name=f"kT{m}")
          for m in range(NM)]
    v8 = [persist.tile([128, NH * DH], F32R, tag=f"v8_{c}", name=f"v8_{c}")
          for c in range(SC)]
    rzall = [persist.tile([128, SC], F32, tag=f"rz{h}", name=f"rz{h}")
             for h in range(NH)]

    hidtv = hid_d.rearrange("(c p) s -> c p s", p=128)

    # --- helper: one (head, sq-chunk) of scores->softmax->weights --------
    def a_iter(ps, wtp, m, hh, sqc, psa_bufs=1):
        h = 2 * m + hh
        hoff = hh * 64
        lq = qT[m][hoff:hoff + 64, sqc * 128:(sqc + 1) * 128]
        wt = wtp.tile([128, S], F32, tag="wt", name="wt")
        zs = []
        for skh in range(2):
            psa = ps.tile([128, 1024], F32, tag="psa", name="psa",
                          bufs=psa_bufs)
            for j in range(2):
                skb = skh * 2 + j
                nc.tensor.matmul(
                    psa[:, j * 512:(j + 1) * 512], lq,
                    kT[m][hoff:hoff + 64, skb * 512:(skb + 1) * 512],
                    start=True, stop=True)
            z = small.tile([128, 1], F32, tag=f"z{skh}", name=f"z{skh}")
            nc.scalar.activation(
                out=wt[:, skh * 1024:(skh + 1) * 1024], in_=psa[:],
                func=AF.Exp, scale=SCALE, accum_out=z[:])
            zs.append(z)
        zt = small.tile([128, 1], F32, tag="zt", name="zt")
        nc.vector.tensor_tensor(out=zt[:], in0=zs[0][:], in1=zs[1][:],
                                op=ADD)
        rz = rzall[h][:, sqc:sqc + 1]
        nc.vector.reciprocal(out=rz, in_=zt[:])
        # normalize on DVE (GpSimd tensor_scalar measured 15.8us/op vs
        # DVE 0.2-0.5us), then one contiguous 1 MB DMA
        nc.vector.tensor_scalar_mul(wt[:], wt[:], rz)
        nc.sync.dma_start(out=w_out[h, sqc * 128:(sqc + 1) * 128, :],
                          in_=wt[:])

    # =====================================================================
    # Prologue: weight rounding, kT pass, then per-oct {qT, v, pair-0 A}.
    # Pair-0's softmax starts as soon as kT is complete and its qT chunk
    # exists, so ACT ramps ~30us in instead of waiting for all projections.
    # =====================================================================
    wpk = tc.alloc_tile_pool(name="wpk", bufs=1)
    wpqv = tc.alloc_tile_pool(name="wpqv", bufs=1)
    wph = tc.alloc_tile_pool(name="wph", bufs=1)
    st1 = tc.alloc_tile_pool(name="st1", bufs=1)
    sb_w0 = tc.alloc_tile_pool(name="sb_w0", bufs=3)
    wst = tc.alloc_tile_pool(name="wst", bufs=2)
    ps1 = tc.alloc_tile_pool(name="ps1", bufs=2, space="PSUM")
    psA0 = tc.alloc_tile_pool(name="psA0", bufs=1, space="PSUM")

    wk_r = wpk.tile([128, EC, NH * DH], F32R, tag="wk_r", name="wk_r")
    wq_r = wpqv.tile([128, EC, NH * DH], F32R, tag="wq_r", name="wq_r")
    wv_r = wpqv.tile([128, EC, NH * DH], F32R, tag="wv_r", name="wv_r")
    for (src_d, dst) in ((wk_d, wk_r), (wq_d, wq_r), (wv_d, wv_r)):
        srcv = src_d.rearrange("(c p) n -> c p n", p=128)
        for c in range(EC):
            st = wst.tile([128, NH * DH], F32, tag="wstage", name="wstage")
            nc.sync.dma_start(out=st[:], in_=srcv[c])
            nc.vector.tensor_copy(out=dst[:, c, :], in_=st[:])

    # pass 1: kT for all pairs
    for oct_ in range(8):
        hT8 = wph.tile([128, EC, 256], F32R, tag="hT8", name="hT8", bufs=2)
        _load_hT_oct(nc, st1, hidtv, oct_, hT8)
        s_lo = oct_ * 256
        for m in range(NM):
            pk = ps1.tile([128, 256], F32, tag="pj", name="pk")
            for ec in range(EC):
                nc.tensor.matmul(pk[:], wk_r[:, ec, m * 128:(m + 1) * 128],
                                 hT8[:, ec, :], start=(ec == 0),
                                 stop=(ec == EC - 1))
            nc.vector.tensor_scalar_add(
                kT[m][:, s_lo:s_lo + 256], pk[:], bk_sb[:, m:m + 1])

    # pass 2: qT + v, with pair-0 A-iters interleaved per oct
    for oct_ in range(8):
        hT8 = wph.tile([128, EC, 256], F32R, tag="hT8", name="hT8", bufs=2)
        _load_hT_oct(nc, st1, hidtv, oct_, hT8)
        s_lo = oct_ * 256
        for m in range(NM):
            pq = ps1.tile([128, 256], F32, tag="pj", name="pq")
            for ec in range(EC):
                nc.tensor.matmul(pq[:], wq_r[:, ec, m * 128:(m + 1) * 128],
                                 hT8[:, ec, :], start=(ec == 0),
                                 stop=(ec == EC - 1))
            nc.vector.tensor_scalar_add(
                qT[m][:, s_lo:s_lo + 256], pq[:], bq_sb[:, m:m + 1])
        for i in range(2):
            sc = oct_ * 2 + i
            pv = ps1.tile([128, 512], F32, tag="pv", name="pv")
            for ec in range(EC):
                nc.tensor.matmul(pv[:], hT8[:, ec, i * 128:(i + 1) * 128],
                                 wv_r[:, ec, :],
                                 start=(ec == 0), stop=(ec == EC - 1))
            nc.vector.tensor_copy(out=v8[sc][:], in_=pv[:])
        for i in range(2):
            sqc = oct_ * 2 + i
            for hh in range(2):
                a_iter(psA0, sb_w0, 0, hh, sqc, psa_bufs=2)

    psA0.release()
    ps1.release()
    wst.release()
    sb_w0.release()
    st1.release()
    wph.release()
    wpqv.release()
    wpk.release()

    # =====================================================================
    # Attention pipeline: T(m) || A(m+1), then T(3) || out-projection.
    # =====================================================================
    pb_attn = tc.alloc_tile_pool(name="pb_attn", bufs=1)
    sb_w = tc.alloc_tile_pool(name="sb_w", bufs=3)
    sb_exp = tc.alloc_tile_pool(name="sb_exp", bufs=2)
    sb_bc = tc.alloc_tile_pool(name="sb_bc", bufs=2)
    attnT = [pb_attn.tile([128, S], F32R, tag=f"attnT{m}", name=f"attnT{m}")
             for m in range(NM)]

    def t_block(ps, m, sqb, filler=None, pt_bufs=1, acc_bufs=2):
        acc0 = ps.tile([64, 512], F32, tag="acc0", name="acc0",
                       bufs=acc_bufs)
        acc1 = ps.tile([64, 512], F32, tag="acc1", name="acc1",
                       bufs=acc_bufs)
        rq0 = qT[m][0:64, sqb * 512:(sqb + 1) * 512]
        rq1 = qT[m][64:128, sqb * 512:(sqb + 1) * 512]
        for skc in range(SC):
            pt = ps.tile([128, 1024], F32, tag="pt", name="pt",
                         bufs=pt_bufs)
            nc.tensor.matmul(pt[:, 0:512],
                             kT[m][0:64, skc * 128:(skc + 1) * 128],
                             rq0, start=True, stop=True,
                             tile_position=(0, 0))
            nc.tensor.matmul(pt[:, 512:1024],
                             kT[m][64:128, skc * 128:(skc + 1) * 128],
                             rq1, start=True, stop=True,
                             tile_position=(64, 0))
            e = sb_exp.tile([128, 1024], F32R, tag="e", name="e")
            nc.scalar.activation(out=e[:], in_=pt[:], func=AF.Exp,
                                 scale=SCALE)
            nc.tensor.matmul(acc0[:], v8[skc][:, m * 128:m * 128 + 64],
                             e[:, 0:512], start=(skc == 0),
                             stop=(skc == SC - 1))
            nc.tensor.matmul(acc1[:],
                             v8[skc][:, m * 128 + 64:m * 128 + 128],
                             e[:, 512:1024], start=(skc == 0),
                             stop=(skc == SC - 1))
            if filler is not None:
                filler(skc)
        for hh, acc in ((0, acc0), (1, acc1)):
            h = 2 * m + hh
            pbc = ps.tile([64, 512], F32, tag="pt", name=f"pbc{hh}",
                          bufs=pt_bufs)
            for c in range(4):
                sqc = sqb * 4 + c
                cb = sb_bc.tile([128, 64], F32, tag="cb", name="cb")
                nc.vector.tensor_scalar_mul(cb[:], ones_t[:],
                                            rzall[h][:, sqc:sqc + 1])
                nc.tensor.transpose(pbc[:, c * 128:(c + 1) * 128], cb[:],
                                    ident[:])
            pbc_sb = sb_bc.tile([64, 512], F32, tag="pbc_sb",
                                name="pbc_sb")
            nc.vector.tensor_copy(out=pbc_sb[:], in_=pbc[:])
            dst = attnT[m][hh * 64:(hh + 1) * 64,
                           sqb * 512:(sqb + 1) * 512]
            nc.vector.tensor_tensor(out=dst, in0=acc[:], in1=pbc_sb[:],
                                    op=MULT)
            nc.vector.tensor_scalar_add(
                dst, dst, bv_sb[hh * 64:(hh + 1) * 64, m:m + 1])

    with tc.tile_pool(name="psAT", bufs=1, space="PSUM") as ps:
        for m in range(NM - 1):
            for sqb in range(4):
                for sqc4 in range(4):
                    sqc = sqb * 4 + sqc4
                    for hh in range(2):
                        a_iter(ps, sb_w, m + 1, hh, sqc)
                t_block(ps, m, sqb)

    # ---- T(3) overlapped with the output projection (lagged 1 block) ----
    with tc.tile_pool(name="psF", bufs=1, space="PSUM") as psF, \
         tc.tile_pool(name="wp3", bufs=1) as wp3, \
         tc.tile_pool(name="st3", bufs=1) as st3:
        wot_r = wp3.tile([128, NM, E], F32R, tag="wot_r", name="wot_r")
        wotv = wot_d.rearrange("(c p) n -> c p n", p=128)
        for c in range(NM):
            st = st3.tile([128, E], F32, tag="wotstage", name="wotstage")
            nc.sync.dma_start(out=st[:], in_=wotv[c])
            nc.vector.tensor_copy(out=wot_r[:, c, :], in_=st[:])

        xt_cur = [None]

        def outproj_piece(sqb, g):
            sqc4, eb = divmod(g, 2)
            sqc = sqb * 4 + sqc4
            if eb == 0:
                xt_cur[0] = sb_w.tile([128, E], F32, tag="wt", name="xt")
            xt = xt_cur[0]
            px = psF.tile([128, 512], F32, tag="px", name="px", bufs=2)
            for cc in range(NM):
                nc.tensor.matmul(
                    px[:], attnT[cc][:, sqc * 128:(sqc + 1) * 128],
                    wot_r[:, cc, eb * 512:(eb + 1) * 512],
                    start=(cc == 0), stop=(cc == NM - 1))
            nc.vector.tensor_copy(out=xt[:, eb * 512:(eb + 1) * 512],
                                  in_=px[:])
            if eb == 1:
                nc.sync.dma_start(out=x_out[sqc * 128:(sqc + 1) * 128, :],
                                  in_=xt[:])

        for sqb in range(4):
            if sqb > 0:
                fill = lambda skc, b=sqb - 1: (
                    outproj_piece(b, skc // 2) if skc % 2 == 1 else None)
            else:
                fill = None
            t_block(psF, 3, sqb, filler=fill, pt_bufs=2, acc_bufs=1)
        for g in range(8):
            outproj_piece(3, g)

    sb_bc.release()
    sb_exp.release()
    sb_w.release()
    pb_attn.release()
    small.release()
    persist.release()


def _build_bass(repeat=1):
    nc = bass.Bass("TRN2", target_bir_lowering=False, debug=False,
                   num_devices=N_CORES)

    hid_d = nc.dram_tensor("hidt", [E, S], F32, kind="ExternalInput").ap()
    wq_d = nc.dram_tensor("wq", [E, NH * DH], F32, kind="ExternalInput").ap()
    wk_d = nc.dram_tensor("wk", [E, NH * DH], F32, kind="ExternalInput").ap()
    wv_d = nc.dram_tensor("wv", [E, NH * DH], F32, kind="ExternalInput").ap()
    bq_d = nc.dram_tensor("bq", [128, NM], F32, kind="ExternalInput").ap()
    bk_d = nc.dram_tensor("bk", [128, NM], F32, kind="ExternalInput").ap()
    bv_d = nc.dram_tensor("bv", [128, NM], F32, kind="ExternalInput").ap()
    wot_d = nc.dram_tensor("wot", [NH * DH, E], F32, kind="ExternalInput").ap()

    w_out = nc.dram_tensor("w_out", [NH, S, S], F32,
                           kind="ExternalOutput").ap()
    x_out = nc.dram_tensor("x_out", [S, E], F32, kind="ExternalOutput").ap()

    with tile.TileContext(nc) as tc:
        for _ in range(repeat):
            _body(nc, tc, hid_d, wq_d, wk_d, wv_d, bq_d, bk_d, bv_d, wot_d,
                  w_out, x_out)
    return nc


_nc_cache = [None]


def _get_nc():
    if _nc_cache[0] is None:
        _install_birfix()
        _nc_cache[0] = _build_bass()
    return _nc_cache[0]


def kernel(hidden_state, Wq, bq, Wk, bk, Wv, bv, Wo, bo, _want_trace=False):
    hidden_state = np.asarray(hidden_state, dtype=np.float32)
    Wq = np.asarray(Wq, dtype=np.float32)
    Wk = np.asarray(Wk, dtype=np.float32)
    Wv = np.asarray(Wv, dtype=np.float32)
    bq = np.asarray(bq, dtype=np.float32)
    bk = np.asarray(bk, dtype=np.float32)
    bv = np.asarray(bv, dtype=np.float32)
    Wo = np.asarray(Wo, dtype=np.float32)
    bo = np.asarray(bo, dtype=np.float32)

    nc = _get_nc()

    in_maps = []
    for core in range(N_CORES):
        b = core // 2
        hg = core % 2
        hs = slice(hg * NH, (hg + 1) * NH)
        # [h, E, DH] -> [E, h*DH]  (head-major feature order)
        wq8 = np.ascontiguousarray(
            Wq[hs].transpose(1, 0, 2).reshape(E, NH * DH))
        wk8 = np.ascontiguousarray(
            Wk[hs].transpose(1, 0, 2).reshape(E, NH * DH))
        wv8 = np.ascontiguousarray(
            Wv[hs].transpose(1, 0, 2).reshape(E, NH * DH))
        # biases as [128, NM]: partition = pair-local c (2 heads x 64 d)
        bq8 = np.ascontiguousarray(bq[hs].reshape(NM, 128).T)
        bk8 = np.ascontiguousarray(bk[hs].reshape(NM, 128).T)
        bv8 = np.ascontiguousarray(bv[hs].reshape(NM, 128).T)
        # Wo column slice, transposed: [c, eo]
        wot = np.ascontiguousarray(Wo[:, hg * 512:(hg + 1) * 512].T)
        in_maps.append({
            "hidt": np.ascontiguousarray(hidden_state[b].T),
            "wq": wq8, "wk": wk8, "wv": wv8,
            "bq": bq8, "bk": bk8, "bv": bv8,
            "wot": wot,
        })

    res = run_bass_kernel_spmd(nc, in_maps, list(range(N_CORES)),
                               trace=_want_trace)

    weights = np.empty((H, B, S, S), dtype=np.float32)
    x = np.zeros((B, S, E), dtype=np.float64)
    for core in range(N_CORES):
        b = core // 2
        hg = core % 2
        r = res.results[core]
        weights[hg * NH:(hg + 1) * NH, b] = r["w_out"]
        x[b] += r["x_out"].astype(np.float64)
    x += bo.astype(np.float64)
    x = x.astype(np.float32)

    # electrode_attention = mean over heads then mean over query dim
    electrode = weights.astype(np.float64).mean(axis=(0, 2)).astype(np.float32)

    if _want_trace:
        kernel._last_result = res
    return x, weights, electrode


# revision 26
# speedup vs baseline: 133.4259x; 1.8569x over previous
"""Trainium2 Bass kernel for nn_MultiHeadAttention_33088428048411.

B=4, S=2048, E=1024, H=16, DH=64.  Outputs: x [B,S,E], weights [H,B,S,S],
electrode_attention [B,S].

Sharding: 8 cores = (batch b in 0..3) x (head-group hg in 0..1); each core owns
one batch element and 8 heads.  Per core, on device: hidden is transposed via
the PE (hT, [e, s]); qT/kT projections land in [d, s] layout and v in [s, d];
scores are computed in both orientations ([sq, sk] for the softmax/weights
output, [sk, sq] for the attn.v contraction); softmax uses exp with the
activation accum_out row-sum (scores are O(1), no max subtraction needed);
the weights output is normalized in place on GpSimd; attn.v accumulates
unnormalized and is renormalized per query via a DVE column-broadcast of 1/Z
transposed on the PE; x_partial = attnT.T @ Wo.T for the core's 8 heads.
Host sums the two x partials per batch, adds bo, and reduces
electrode_attention from the returned weights.

All matmuls run in float32r (fp32 operands rounded by the producing DVE op;
measured ~1.4e-4 max rel err vs ~2.3e-3 for bf16, at full PE speed for
moving dims >= 256).
"""

import numpy as np
import orjson

import concourse.bass as bass
import concourse.mybir as mybir
import concourse.tile as tile
from concourse.masks import make_identity
from concourse.bass_utils import run_bass_kernel_spmd

F32 = mybir.dt.float32
F32R = mybir.dt.float32r
AF = mybir.ActivationFunctionType
MULT = mybir.AluOpType.mult
ADD = mybir.AluOpType.add

B, S, E, H, DH = 4, 2048, 1024, 16, 64
NH = 8          # heads per core
NM = NH // 2    # head pairs per core
EC = E // 128   # 8 contraction chunks
SC = S // 128   # 16 s chunks
N_CORES = 8
SCALE = 1.0 / 8.0  # 1/sqrt(DH)

# ---------------------------------------------------------------------------
# Walrus in this container rejects instructions carrying more than one sync
# wait ("Too many sync wait commands" -- the fused Matmult word has a single
# wait slot).  Tile's sem assignment attaches several.  Fix at the BIR-JSON
# level: every instruction keeps its last wait; the rest move to NoOps
# inserted immediately before it on the same engine.
_wsplit_counter = [0]


def _split_waits(module):
    for fn in module.get("functions", []):
        for bb in fn.get("blocks", []):
            out = []
            for inst in bb.get("instructions", []):
                si = inst.get("sync_info")
                waits = si.get("on_wait") if si else None
                if waits and len(waits) > 1:
                    excess, keep = waits[:-1], waits[-1:]
                    for w in excess:
                        _wsplit_counter[0] += 1
                        out.append({
                            "debug": inst.get("debug", 0),
                            "engine": inst["engine"],
                            "ins": [],
                            "name": f"{inst['name']}-ws{_wsplit_counter[0]}",
                            "opcode": "NoOp",
                            "outs": [],
                            "sync_info": {"on_update": [], "on_wait": [w]},
                        })
                    si["on_wait"] = keep
                out.append(inst)
            bb["instructions"] = out
    return module


def _install_birfix():
    if getattr(bass.Bass, "_birfix_installed", False):
        return
    orig = bass.Bass.to_json_bytes

    def to_json_bytes(self):
        return orjson.dumps(_split_waits(orjson.loads(orig(self))))

    bass.Bass.to_json_bytes = to_json_bytes
    bass.Bass._birfix_installed = True


# ---------------------------------------------------------------------------
def _load_hT_oct(nc, st_pool, hidtv, oct_, hT8):
    """Fill hT8 [128, EC, 256] from the host-transposed hidden (f32r round
    on DVE).  hidtv is hidt viewed as [c, p, s]."""
    hstage = st_pool.tile([128, EC, 256], F32, tag="hstage", name="hstage")
    nc.sync.dma_start(
        out=hstage[:],
        in_=hidtv[:, :, oct_ * 256:(oct_ + 1) * 256].rearrange(
            "c p s -> p c s"))
    nc.vector.tensor_copy(out=hT8[:], in_=hstage[:])


def _body(nc, tc, hid_d, wq_d, wk_d, wv_d, bq_d, bk_d, bv_d, wot_d,
          w_out, x_out, skip_w_dma=False):
    persist = tc.alloc_tile_pool(name="persist", bufs=1)
    small = tc.alloc_tile_pool(name="small", bufs=4)

    # --- constants -------------------------------------------------------
    ident = persist.tile([128, 128], F32, tag="ident", name="ident")
    make_identity(nc, ident[:])
    ones_t = persist.tile([128, 64], F32, tag="ones_t", name="ones_t")
    nc.vector.memset(ones_t[:], 1.0)

    bq_sb = persist.tile([128, NM], F32, tag="bq_sb", name="bq_sb")
    bk_sb = persist.tile([128, NM], F32, tag="bk_sb", name="bk_sb")
    bv_sb = persist.tile([128, NM], F32, tag="bv_sb", name="bv_sb")
    nc.sync.dma_start(out=bq_sb[:], in_=bq_d)
    nc.sync.dma_start(out=bk_sb[:], in_=bk_d)
    nc.sync.dma_start(out=bv_sb[:], in_=bv_d)

    # --- persistent activations -----------------------------------------
    qT = [persist.tile([128, S], F32R, tag=f"qT{m}", name=f"qT{m}")
          for m in range(NM)]
    kT = [persist.tile([128, S], F32R, tag=f"kT{m}", name=f"kT{m}")
          for m in range(NM)]
    v8 = [persist.tile([128, NH * DH], F32R, tag=f"v8_{c}", name=f"v8_{c}")
          for c in range(SC)]
    rzall = [persist.tile([128, SC], F32, tag=f"rz{h}", name=f"rz{h}")
             for h in range(NH)]

    hidtv = hid_d.rearrange("(c p) s -> c p s", p=128)

    # --- helper: one (head, sq-chunk) of scores->softmax->weights --------
    def a_iter(ps, wtp, m, hh, sqc, psa_bufs=1):
        h = 2 * m + hh
        hoff = hh * 64
        lq = qT[m][hoff:hoff + 64, sqc * 128:(sqc + 1) * 128]
        wt = wtp.tile([128, S], F32, tag="wt", name="wt")
        zs = []
        for skh in range(2):
            psa = ps.tile([128, 1024], F32, tag="psa", name="psa",
                          bufs=psa_bufs)
            for j in range(2):
                skb = skh * 2 + j
                nc.tensor.matmul(
                    psa[:, j * 512:(j + 1) * 512], lq,
                    kT[m][hoff:hoff + 64, skb * 512:(skb + 1) * 512],
                    start=True, stop=True)
            z = small.tile([128, 1], F32, tag=f"z{skh}", name=f"z{skh}")
            nc.scalar.activation(
                out=wt[:, skh * 1024:(skh + 1) * 1024], in_=psa[:],
                func=AF.Exp, scale=SCALE, accum_out=z[:])
            zs.append(z)
        zt = small.tile([128, 1], F32, tag="zt", name="zt")
        nc.vector.tensor_tensor(out=zt[:], in0=zs[0][:], in1=zs[1][:],
                                op=ADD)
        rz = rzall[h][:, sqc:sqc + 1]
        nc.vector.reciprocal(out=rz, in_=zt[:])
        # normalize on DVE (GpSimd tensor_scalar measured 15.8us/op vs
        # DVE 0.2-0.5us), then one contiguous 1 MB DMA
        nc.vector.tensor_scalar_mul(wt[:], wt[:], rz)
        if not skip_w_dma:
            nc.sync.dma_start(out=w_out[h, sqc * 128:(sqc + 1) * 128, :],
                              in_=wt[:])

    # =====================================================================
    # Prologue: weight rounding, kT pass, then per-oct {qT, v, pair-0 A}.
    # Pair-0's softmax starts as soon as kT is complete and its qT chunk
    # exists, so ACT ramps ~30us in instead of waiting for all projections.
    # =====================================================================
    wpk = tc.alloc_tile_pool(name="wpk", bufs=1)
    wpqv = tc.alloc_tile_pool(name="wpqv", bufs=1)
    wph = tc.alloc_tile_pool(name="wph", bufs=1)
    st1 = tc.alloc_tile_pool(name="st1", bufs=1)
    sb_w0 = tc.alloc_tile_pool(name="sb_w0", bufs=3)
    wst = tc.alloc_tile_pool(name="wst", bufs=2)
    ps1 = tc.alloc_tile_pool(name="ps1", bufs=2, space="PSUM")
    psA0 = tc.alloc_tile_pool(name="psA0", bufs=1, space="PSUM")

    wk_r = wpk.tile([128, EC, NH * DH], F32R, tag="wk_r", name="wk_r")
    wq_r = wpqv.tile([128, EC, NH * DH], F32R, tag="wq_r", name="wq_r")
    wv_r = wpqv.tile([128, EC, NH * DH], F32R, tag="wv_r", name="wv_r")
    for (src_d, dst) in ((wk_d, wk_r), (wq_d, wq_r), (wv_d, wv_r)):
        srcv = src_d.rearrange("(c p) n -> c p n", p=128)
        for c in range(EC):
            st = wst.tile([128, NH * DH], F32, tag="wstage", name="wstage")
            nc.sync.dma_start(out=st[:], in_=srcv[c])
            nc.vector.tensor_copy(out=dst[:, c, :], in_=st[:])

    # pass 1: kT for all pairs
    for oct_ in range(8):
        hT8 = wph.tile([128, EC, 256], F32R, tag="hT8", name="hT8", bufs=2)
        _load_hT_oct(nc, st1, hidtv, oct_, hT8)
        s_lo = oct_ * 256
        for m in range(NM):
            pk = ps1.tile([128, 256], F32, tag="pj", name="pk")
            for ec in range(EC):
                nc.tensor.matmul(pk[:], wk_r[:, ec, m * 128:(m + 1) * 128],
                                 hT8[:, ec, :], start=(ec == 0),
                                 stop=(ec == EC - 1))
            nc.vector.tensor_scalar_add(
                kT[m][:, s_lo:s_lo + 256], pk[:], bk_sb[:, m:m + 1])

    # pass 2: qT + v, with pair-0 A-iters interleaved per oct
    for oct_ in range(8):
        hT8 = wph.tile([128, EC, 256], F32R, tag="hT8", name="hT8", bufs=2)
        _load_hT_oct(nc, st1, hidtv, oct_, hT8)
        s_lo = oct_ * 256
        for m in range(NM):
            pq = ps1.tile([128, 256], F32, tag="pj", name="pq")
            for ec in range(EC):
                nc.tensor.matmul(pq[:], wq_r[:, ec, m * 128:(m + 1) * 128],
                                 hT8[:, ec, :], start=(ec == 0),
                                 stop=(ec == EC - 1))
            nc.vector.tensor_scalar_add(
                qT[m][:, s_lo:s_lo + 256], pq[:], bq_sb[:, m:m + 1])
        for i in range(2):
            sc = oct_ * 2 + i
            pv = ps1.tile([128, 512], F32, tag="pv", name="pv")
            for ec in range(EC):
                nc.tensor.matmul(pv[:], hT8[:, ec, i * 128:(i + 1) * 128],
                                 wv_r[:, ec, :],
                                 start=(ec == 0), stop=(ec == EC - 1))
            nc.vector.tensor_copy(out=v8[sc][:], in_=pv[:])
        for i in range(2):
            sqc = oct_ * 2 + i
            for hh in range(2):
                a_iter(psA0, sb_w0, 0, hh, sqc, psa_bufs=2)

    psA0.release()
    ps1.release()
    wst.release()
    sb_w0.release()
    st1.release()
    wph.release()
    wpqv.release()
    wpk.release()

    # =====================================================================
    # Attention pipeline: T(m) || A(m+1), then T(3) || out-projection.
    # =====================================================================
    pb_attn = tc.alloc_tile_pool(name="pb_attn", bufs=1)
    sb_w = tc.alloc_tile_pool(name="sb_w", bufs=3)
    sb_exp = tc.alloc_tile_pool(name="sb_exp", bufs=2)
    sb_bc = tc.alloc_tile_pool(name="sb_bc", bufs=2)
    attnT = [pb_attn.tile([128, S], F32R, tag=f"attnT{m}", name=f"attnT{m}")
             for m in range(NM)]

    def t_block(ps, m, sqb, filler=None, pt_bufs=1, acc_bufs=2):
        acc0 = ps.tile([64, 512], F32, tag="acc0", name="acc0",
                       bufs=acc_bufs)
        acc1 = ps.tile([64, 512], F32, tag="acc1", name="acc1",
                       bufs=acc_bufs)
        rq0 = qT[m][0:64, sqb * 512:(sqb + 1) * 512]
        rq1 = qT[m][64:128, sqb * 512:(sqb + 1) * 512]
        for skc in range(SC):
            pt = ps.tile([128, 1024], F32, tag="pt", name="pt",
                         bufs=pt_bufs)
            nc.tensor.matmul(pt[:, 0:512],
                             kT[m][0:64, skc * 128:(skc + 1) * 128],
                             rq0, start=True, stop=True,
                             tile_position=(0, 0))
            nc.tensor.matmul(pt[:, 512:1024],
                             kT[m][64:128, skc * 128:(skc + 1) * 128],
                             rq1, start=True, stop=True,
                             tile_position=(64, 0))
            e = sb_exp.tile([128, 1024], F32R, tag="e", name="e")
            nc.scalar.activation(out=e[:], in_=pt[:], func=AF.Exp,
                                 scale=SCALE)
            nc.tensor.matmul(acc0[:], v8[skc][:, m * 128:m * 128 + 64],
                             e[:, 0:512], start=(skc == 0),
                             stop=(skc == SC - 1))
            nc.tensor.matmul(acc1[:],
                             v8[skc][:, m * 128 + 64:m * 128 + 128],
                             e[:, 512:1024], start=(skc == 0),
                             stop=(skc == SC - 1))
            if filler is not None:
                filler(skc)
        for hh, acc in ((0, acc0), (1, acc1)):
            h = 2 * m + hh
            pbc = ps.tile([64, 512], F32, tag="pt", name=f"pbc{hh}",
                          bufs=pt_bufs)
            for c in range(4):
                sqc = sqb * 4 + c
                cb = sb_bc.tile([128, 64], F32, tag="cb", name="cb")
                nc.vector.tensor_scalar_mul(cb[:], ones_t[:],
                                            rzall[h][:, sqc:sqc + 1])
                nc.tensor.transpose(pbc[:, c * 128:(c + 1) * 128], cb[:],
                                    ident[:])
            pbc_sb = sb_bc.tile([64, 512], F32, tag="pbc_sb",
                                name="pbc_sb")
            nc.vector.tensor_copy(out=pbc_sb[:], in_=pbc[:])
            dst = attnT[m][hh * 64:(hh + 1) * 64,
                           sqb * 512:(sqb + 1) * 512]
            nc.vector.tensor_tensor(out=dst, in0=acc[:], in1=pbc_sb[:],
                                    op=MULT)
            nc.vector.tensor_scalar_add(
                dst, dst, bv_sb[hh * 64:(hh + 1) * 64, m:m + 1])

    with tc.tile_pool(name="psAT", bufs=1, space="PSUM") as ps:
        for m in range(NM - 1):
            for sqb in range(4):
                for sqc4 in range(4):
                    sqc = sqb * 4 + sqc4
                    for hh in range(2):
                        a_iter(ps, sb_w, m + 1, hh, sqc)
                t_block(ps, m, sqb)

    # ---- T(3) overlapped with the output projection (lagged 1 block) ----
    with tc.tile_pool(name="psF", bufs=1, space="PSUM") as psF, \
         tc.tile_pool(name="wp3", bufs=1) as wp3, \
         tc.tile_pool(name="st3", bufs=1) as st3:
        wot_r = wp3.tile([128, NM, E], F32R, tag="wot_r", name="wot_r")
        wotv = wot_d.rearrange("(c p) n -> c p n", p=128)
        for c in range(NM):
            st = st3.tile([128, E], F32, tag="wotstage", name="wotstage")
            nc.sync.dma_start(out=st[:], in_=wotv[c])
            nc.vector.tensor_copy(out=wot_r[:, c, :], in_=st[:])

        xt_cur = [None]

        def outproj_piece(sqb, g):
            sqc4, eb = divmod(g, 2)
            sqc = sqb * 4 + sqc4
            if eb == 0:
                xt_cur[0] = sb_w.tile([128, E], F32, tag="wt", name="xt")
            xt = xt_cur[0]
            px = psF.tile([128, 512], F32, tag="px", name="px", bufs=2)
            for cc in range(NM):
                nc.tensor.matmul(
                    px[:], attnT[cc][:, sqc * 128:(sqc + 1) * 128],
                    wot_r[:, cc, eb * 512:(eb + 1) * 512],
                    start=(cc == 0), stop=(cc == NM - 1))
            nc.vector.tensor_copy(out=xt[:, eb * 512:(eb + 1) * 512],
                                  in_=px[:])
            if eb == 1:
                nc.sync.dma_start(out=x_out[sqc * 128:(sqc + 1) * 128, :],
                                  in_=xt[:])

        for sqb in range(4):
            if sqb > 0:
                fill = lambda skc, b=sqb - 1: (
                    outproj_piece(b, skc // 2) if skc % 2 == 1 else None)
            else:
                fill = None
            t_block(psF, 3, sqb, filler=fill, pt_bufs=2, acc_bufs=1)
        for g in range(8):
            outproj_piece(3, g)

    sb_bc.release()
    sb_exp.release()
    sb_w.release()
    pb_attn.release()
    small.release()
    persist.release()


def _build_bass(repeat=1, skip_w_dma=False):
    nc = bass.Bass("TRN2", target_bir_lowering=False, debug=False,
                   num_devices=N_CORES)

    hid_d = nc.dram_tensor("hidt", [E, S], F32, kind="ExternalInput").ap()
    wq_d = nc.dram_tensor("wq", [E, NH * DH], F32, kind="ExternalInput").ap()
    wk_d = nc.dram_tensor("wk", [E, NH * DH], F32, kind="ExternalInput").ap()
    wv_d = nc.dram_tensor("wv", [E, NH * DH], F32, kind="ExternalInput").ap()
    bq_d = nc.dram_tensor("bq", [128, NM], F32, kind="ExternalInput").ap()
    bk_d = nc.dram_tensor("bk", [128, NM], F32, kind="ExternalInput").ap()
    bv_d = nc.dram_tensor("bv", [128, NM], F32, kind="ExternalInput").ap()
    wot_d = nc.dram_tensor("wot", [NH * DH, E], F32, kind="ExternalInput").ap()

    w_out = nc.dram_tensor("w_out", [NH, S, S], F32,
                           kind="ExternalOutput").ap()
    x_out = nc.dram_tensor("x_out", [S, E], F32, kind="ExternalOutput").ap()

    with tile.TileContext(nc) as tc:
        for _ in range(repeat):
            _body(nc, tc, hid_d, wq_d, wk_d, wv_d, bq_d, bk_d, bv_d, wot_d,
                  w_out, x_out, skip_w_dma=skip_w_dma)
    return nc


_nc_cache = [None]


def _get_nc():
    if _nc_cache[0] is None:
        _install_birfix()
        _nc_cache[0] = _build_bass()
    return _nc_cache[0]


def kernel(hidden_state, Wq, bq, Wk, bk, Wv, bv, Wo, bo, _want_trace=False):
    hidden_state = np.asarray(hidden_state, dtype=np.float32)
    Wq = np.asarray(Wq, dtype=np.float32)
    Wk = np.asarray(Wk, dtype=np.float32)
    Wv = np.asarray(Wv, dtype=np.float32)
    bq = np.asarray(bq, dtype=np.float32)
    bk = np.asarray(bk, dtype=np.float32)
    bv = np.asarray(bv, dtype=np.float32)
    Wo = np.asarray(Wo, dtype=np.float32)
    bo = np.asarray(bo, dtype=np.float32)

    nc = _get_nc()

    in_maps = []
    for core in range(N_CORES):
        b = core // 2
        hg = core % 2
        hs = slice(hg * NH, (hg + 1) * NH)
        # [h, E, DH] -> [E, h*DH]  (head-major feature order)
        wq8 = np.ascontiguousarray(
            Wq[hs].transpose(1, 0, 2).reshape(E, NH * DH))
        wk8 = np.ascontiguousarray(
            Wk[hs].transpose(1, 0, 2).reshape(E, NH * DH))
        wv8 = np.ascontiguousarray(
            Wv[hs].transpose(1, 0, 2).reshape(E, NH * DH))
        # biases as [128, NM]: partition = pair-local c (2 heads x 64 d)
        bq8 = np.ascontiguousarray(bq[hs].reshape(NM, 128).T)
        bk8 = np.ascontiguousarray(bk[hs].reshape(NM, 128).T)
        bv8 = np.ascontiguousarray(bv[hs].reshape(NM, 128).T)
        # Wo column slice, transposed: [c, eo]
        wot = np.ascontiguousarray(Wo[:, hg * 512:(hg + 1) * 512].T)
        in_maps.append({
            "hidt": np.ascontiguousarray(hidden_state[b].T),
            "wq": wq8, "wk": wk8, "wv": wv8,
            "bq": bq8, "bk": bk8, "bv": bv8,
            "wot": wot,
        })

    res = run_bass_kernel_spmd(nc, in_maps, list(range(N_CORES)),
                               trace=_want_trace)

    weights = np.empty((H, B, S, S), dtype=np.float32)
    x = np.zeros((B, S, E), dtype=np.float64)
    for core in range(N_CORES):
        b = core // 2
        hg = core % 2
        r = res.results[core]
        weights[hg * NH:(hg + 1) * NH, b] = r["w_out"]
        x[b] += r["x_out"].astype(np.float64)
    x += bo.astype(np.float64)
    x = x.astype(np.float32)

    # electrode_attention = mean over heads then mean over query dim
    electrode = weights.astype(np.float64).mean(axis=(0, 2)).astype(np.float32)

    if _want_trace:
        kernel._last_result = res
    return x, weights, electrode
